# revision 25
# baseline (speedup 1.0000x reference)
"""HAN (hypergraph attention network) Trainium2 kernel, v2.

Data-parallel over batch: 8 cores x 16 batch elements, params replicated.
v2 pipeline: per-core vocabulary compaction (int16 idx) + bf16 padded
embedding table with a built-in ones column -> dma_gather(transpose=True)
lands activations k-major with zero PE transposes -> bf16 projections with
bias folded into the weight row for the ones column -> bilinear attention
(heads*queries = 128 partitions) -> softmax -> bf16 attention-value matmul
-> pooled -> fc -> candidate sim -> log_softmax.  fc/glove weights are
loaded as a few large resident tiles so the tail phases never stall on DMA.
"""

import numpy as np
import ml_dtypes
from contextlib import ExitStack

import concourse.bass as bass
import concourse.bacc as bacc
import concourse.tile as tile
from concourse import mybir
from concourse.bass_utils import run_bass_kernel_spmd

F32 = mybir.dt.float32
BF16 = mybir.dt.bfloat16
I16 = mybir.dt.int16
AF = mybir.ActivationFunctionType
ALU = mybir.AluOpType
AX = mybir.AxisListType

NCORES = 8
B = 128
BPC = B // NCORES          # 16 batch elems per core
NQ, NS, NODES = 16, 256, 3
V, E = 50000, 300
ES = 384                   # padded emb row (bf16) -> 768B, %256==0
UMAX = 13056               # per-core unique rows cap: 16*(256+16)*3 / ... hard bound
C, H, OUT, NA = 1024, 8, 300, 5000
CC = C // 128              # 8 c-chunks
NCH = 9                    # (node j, k-chunk c) pairs: 3x3
OCN = [128, 128, 44]       # OUT=300 -> 3 o-chunks
SIMCH = [512] * 9 + [392]  # NA=5000 N-chunks
FCT = 4                    # fcw resident tiles
FCC = H * CC // FCT        # (h,cc) chunks per fcw tile

_CACHED = None


def _emit(ctx, tc, ins, outs):
    nc = tc.nc

    # single merged input (fewer per-call args = less dispatch overhead).
    # rows 0:UMAX = emb table (gather); rows UMAX: = a [128, 53982] bf16
    # param block flattened row-major, cols: 0:9216 qwT | 9216:18432 kwT |
    # 18432:18560 idbf | 18560:37760 fcwT | 37760:52760 glo | 52760:53166
    # f32 consts (h2aT|fcb|sel1|sel2, bitcast) | 53166:53982 i16 idx (q|kg)
    allt = ins["all"]           # [UMAX + 128*53982/ES, ES] bf16
    emb = allt[0:UMAX, :]
    wb = allt[UMAX:, :].rearrange("r c -> (r c)").rearrange(
        "(p x) -> p x", p=128)             # [128, 53982] bf16
    wf = wb[:, 52760:53166].bitcast(F32)   # [128, 203] f32
    wi = wb[:, 53166:53982].bitcast(I16)   # [128, 816] i16
    out_d = outs["out"]         # [BPC, NA] f32

    const = ctx.enter_context(tc.tile_pool(name="const", bufs=1))
    actp = ctx.enter_context(tc.tile_pool(name="actp", bufs=2))
    hstp = ctx.enter_context(tc.tile_pool(name="hstp", bufs=2))
    hsbp = ctx.enter_context(tc.tile_pool(name="hsbp", bufs=2))
    xtp = ctx.enter_context(tc.tile_pool(name="xtp", bufs=2))
    attp = ctx.enter_context(tc.tile_pool(name="attp", bufs=2))
    tmpp = ctx.enter_context(tc.tile_pool(name="tmpp", bufs=2))
    smlp = ctx.enter_context(tc.tile_pool(name="smlp", bufs=2))

    pstr = ctx.enter_context(tc.tile_pool(name="pstr", bufs=3, space="PSUM"))
    pspj = ctx.enter_context(tc.tile_pool(name="pspj", bufs=3, space="PSUM"))
    psyt = ctx.enter_context(tc.tile_pool(name="psyt", bufs=2, space="PSUM"))

    # ---- resident constants / weights (sliced out of the merged bufs) ----
    itq = const.tile([128, 48], I16, tag="itq")
    nc.sync.dma_start(itq[:], wi[:, 0:48])
    itk = const.tile([128, 8 * 96], I16, tag="itk")
    nc.sync.dma_start(itk[:], wi[:, 48:816])
    # qwT in 3 sub-tiles so the q-projection starts before the full load
    qwT3 = []
    for s in range(3):
        qt = const.tile([128, 3 * 1024], BF16, tag=f"qwT{s}")
        nc.sync.dma_start(qt[:], wb[:, s * 3072:(s + 1) * 3072])
        qwT3.append(qt)
    kwT3 = []
    for s in range(3):
        kt = const.tile([128, 3 * 1024], BF16, tag=f"kwT{s}")
        nc.sync.dma_start(kt[:], wb[:, 9216 + s * 3072: 9216 + (s + 1) * 3072])
        kwT3.append(kt)
    h2aT = const.tile([128, CC * H], F32, tag="h2aT")
    nc.sync.dma_start(h2aT[:], wf[:, 0:64])
    fcb = const.tile([128, 3], F32, tag="fcb")
    nc.sync.dma_start(fcb[:], wf[:, 64:67])
    sel1 = const.tile([128, H], F32, tag="sel1")
    nc.sync.dma_start(sel1[:], wf[:, 67:75])
    sel2 = const.tile([H, 128], F32, tag="sel2")
    nc.sync.dma_start(sel2[:], wf[0:H, 75:203])
    idbf = const.tile([128, 128], BF16, tag="idbf")
    nc.sync.dma_start(idbf[:], wb[:, 18432:18560])
    fcw_sb = []
    for t in range(FCT):
        fw = const.tile([128, FCC * OUT], BF16, tag=f"fcw{t}")
        nc.sync.dma_start(
            fw[:], wb[:, 18560 + t * FCC * OUT: 18560 + (t + 1) * FCC * OUT])
        fcw_sb.append(fw)
    glo_sb = []
    for oc in range(3):
        gs = const.tile([128, NA], BF16, tag=f"glo{oc}")
        nc.sync.dma_start(gs[:], wb[:, 37760 + oc * NA: 37760 + (oc + 1) * NA])
        glo_sb.append(gs)

    hqT = const.tile([128, CC * 256], F32, tag="hqT")      # [c, b*16+q]
    POOL = const.tile([128, CC * BPC * H], F32, tag="POOL")  # col cc*128+b*8+h
    POOLb = const.tile([128, CC * BPC * H], BF16, tag="POOLb")
    fcout = const.tile([128, 3 * BPC], BF16, tag="fcout")
    sim_sb = const.tile([BPC, NA], F32, tag="sim_sb")
    parti = const.tile([BPC, 16], F32, tag="parti")
    lse = const.tile([BPC, 1], F32, tag="lse")
    tot = const.tile([BPC, 1], F32, tag="tot")

    def gather(idx_slice, ntok):
        """ntok tokens x NODES rows, j-major idx order; num_idxs > 768
        crashes the gather ucode, so one 768-idx gather per contiguous
        [3, 768] block. Returns list of [128, 3, 768] views."""
        ni = NODES * ntok
        ng = ni // 768
        at = actp.tile([128, 3 * ni], BF16, tag="act")
        views = []
        for g in range(ng):
            v = at[:, g * 2304:(g + 1) * 2304].rearrange(
                "p (c n) -> p c n", c=3)
            nc.gpsimd.dma_gather(
                out_ap=v,
                in_ap=emb[:],
                idxs_ap=idx_slice[:, g * 48:(g + 1) * 48],
                num_idxs=768,
                num_idxs_reg=768,
                elem_size=ES,
                transpose=True,
            )
            views.append(v)
        return views

    def project(wT, atvs, dstT, ntok):
        """dstT[:, cc*ntok : +ntok] = wT.T @ act (+bias via ones column).

        rhs pieces per (node j, k-chunk c): with one gather (ntok=256) the
        j-blocks are whole; with two (ntok=512) node 1 straddles the two
        gather blocks, so it contributes two half-range matmuls."""
        if len(atvs) == 1:
            pieces = [(j * 3 + c, atvs[0][:, c, j * ntok:(j + 1) * ntok], 0, ntok)
                      for j in range(3) for c in range(3)]
        else:
            pieces = (
                [(c, atvs[0][:, c, 0:512], 0, 512) for c in range(3)]
                + [(3 + c, atvs[0][:, c, 512:768], 0, 256) for c in range(3)]
                + [(3 + c, atvs[1][:, c, 0:256], 256, 512) for c in range(3)]
                + [(6 + c, atvs[1][:, c, 256:768], 0, 512) for c in range(3)]
            )
            # full-range ops must open and close the accumulation group
            pieces = pieces[0:3] + pieces[3:9] + pieces[9:12]
        for cc in range(CC):
            ps = pspj.tile([128, 512], F32, tag="pjps")
            for i, (ch, rhs, a, bnd) in enumerate(pieces):
                if isinstance(wT, list):
                    w, col = wT[ch // 3], (ch % 3) * 1024 + cc * 128
                else:
                    w, col = wT, ch * 1024 + cc * 128
                nc.tensor.matmul(
                    out=ps[:, a:bnd],
                    lhsT=w[:, col: col + 128],
                    rhs=rhs,
                    start=(i == 0),
                    stop=(i == len(pieces) - 1),
                )
            nc.scalar.copy(out=dstT[:, cc * ntok:(cc + 1) * ntok],
                           in_=ps[:, 0:ntok])

    # ---- prologue: hq for all 16 b (256 ques tokens) ----
    atq = gather(itq[:], 256)
    project(qwT3, atq, hqT, 256)

    hqv = hqT[:].rearrange("p (c t) -> p c t", c=CC)  # [128, 8, 256]
    h2av = h2aT[:].rearrange("p (c h) -> p c h", c=CC)  # [128, 8, 8]
    pv = POOL[:].rearrange("p (c b h) -> p c b h", c=CC, b=BPC)
    pbv = POOLb[:].rearrange("p (c b h) -> p c b h", c=CC, b=BPC)

    # ---- per pair of batch elements ----
    for bp in range(BPC // 2):
        atk = gather(itk[:, bp * 96:(bp + 1) * 96], 512)

        hsT = hstp.tile([128, CC * 512], BF16, tag="hsT")
        project(kwT3, atk, hsT, 512)

        for half in range(2):
            b = bp * 2 + half
            hb = half * 256

            # hs token-major bf16: [s-chunk partitions, col st*1024 + c]
            hs_sb = hsbp.tile([128, 2 * 1024], BF16, tag="hs_sb")
            for st in range(2):
                ps = pstr.tile([128, 1024], BF16, tag="trps")
                for cc in range(CC):
                    nc.tensor.transpose(
                        out=ps[:, cc * 128:(cc + 1) * 128],
                        in_=hsT[:, cc * 512 + hb + st * 128: cc * 512 + hb + st * 128 + 128],
                        identity=idbf[:],
                    )
                nc.scalar.copy(out=hs_sb[:, st * 1024:(st + 1) * 1024], in_=ps[:])

            # X^T[c, h*16+q] = hqT[c, q] * h2aT[c, h]
            XT = xtp.tile([128, 1024], BF16, tag="XT")
            nc.vector.tensor_tensor(
                out=XT[:].rearrange("p (c h q) -> p c h q", c=CC, h=H),
                in0=hqv[:, :, b * 16: b * 16 + 16].unsqueeze(2).to_broadcast(
                    [128, CC, H, 16]),
                in1=h2av[:, :, :].unsqueeze(3).to_broadcast([128, CC, H, 16]),
                op=ALU.mult,
            )

            # logits[hq=128, s=256]
            plg = pspj.tile([128, 512], F32, tag="pjps")
            for cc in range(CC):
                nc.tensor.matmul(
                    out=plg[:, 0:256],
                    lhsT=XT[:, cc * 128: cc * 128 + 128],
                    rhs=hsT[:, cc * 512 + hb: cc * 512 + hb + 256],
                    start=(cc == 0),
                    stop=(cc == CC - 1),
                )

            # softmax over flat (q,s) per (b,h); logits tiny -> skip max-sub
            att = attp.tile([128, 256], BF16, tag="att")
            qsum = smlp.tile([128, 1], F32, tag="qsum")
            nc.scalar.activation(att[:], plg[:, 0:256], AF.Exp, accum_out=qsum[:])

            dps = psyt.tile([128, 512], F32, tag="ytps", name="dps")
            nc.tensor.matmul(out=dps[0:8, 0:1], lhsT=sel1[:], rhs=qsum[:],
                             start=True, stop=True)
            r8 = smlp.tile([8, 1], F32, tag="r8")
            nc.vector.reciprocal(r8[:], dps[0:8, 0:1])
            nc.tensor.matmul(out=dps[:, 1:2], lhsT=sel2[:], rhs=r8[:],
                             start=True, stop=True)
            rsb = smlp.tile([128, 1], F32, tag="rsb")
            nc.vector.tensor_copy(rsb[:], dps[:, 1:2])

            attn = attp.tile([128, 256], BF16, tag="attn")
            nc.vector.tensor_scalar_mul(attn[:], att[:], rsb[:])

            # attT [s, hq] bf16
            attT = attp.tile([128, 256], BF16, tag="attT")
            psTb = pstr.tile([128, 256], BF16, tag="trps", name="psTb")
            for st in range(2):
                nc.tensor.transpose(
                    out=psTb[:, st * 128:(st + 1) * 128],
                    in_=attn[:, st * 128:(st + 1) * 128],
                    identity=idbf[:],
                )
            nc.vector.tensor_copy(attT[:], psTb[:])

            # YT[c, hq] per c-chunk; pooled[h,c] = sum_q hqT * sum_s attT*hs
            for ccg in range(2):
                py = psyt.tile([128, 512], F32, tag="ytps")
                for i in range(4):
                    cc = ccg * 4 + i
                    for st in range(2):
                        nc.tensor.matmul(
                            out=py[:, i * 128:(i + 1) * 128],
                            lhsT=hs_sb[:, st * 1024 + cc * 128: st * 1024 + cc * 128 + 128],
                            rhs=attT[:, st * 128:(st + 1) * 128],
                            start=(st == 0),
                            stop=(st == 1),
                        )
                tmp = tmpp.tile([128, 512], F32, tag="tmp")
                nc.vector.tensor_tensor(
                    out=tmp[:].rearrange("p (c h q) -> p c h q", c=4, h=H),
                    in0=py[:].rearrange("p (c h q) -> p c h q", c=4, h=H),
                    in1=hqv[:, ccg * 4:(ccg + 1) * 4, b * 16: b * 16 + 16].unsqueeze(2).to_broadcast([128, 4, H, 16]),
                    op=ALU.mult,
                )
                nc.vector.reduce_sum(
                    out=pv[:, ccg * 4:(ccg + 1) * 4, b, :],
                    in_=tmp[:].rearrange("p (c h q) -> p c h q", c=4, h=H),
                    axis=AX.X,
                )
            # per-b bf16 conversion keeps the bulk copy off fc's critical path
            nc.vector.tensor_copy(pbv[:, :, b, :], pv[:, :, b, :])

    # ---- fc: out[o, b] = sum_{h,c} fc_w[o, h*1024+c] * pooled ----
    poolv = POOLb[:].rearrange("p (c b h) -> p c b h", c=CC, b=BPC)
    # pfc banks come from pools released by the projections (~175us), not
    # psyt which the last pair's YT holds until ~183us
    pfc = [pspj.tile([128, 512], F32, tag="pjps", name="pfc0"),
           pstr.tile([128, 512], F32, tag="trps", name="pfc1"),
           pspj.tile([128, 512], F32, tag="pjps", name="pfc2")]
    nhc = H * CC
    for h in range(H):
        for cc in range(CC):
            i = h * CC + cc
            fw = fcw_sb[i // FCC]
            fo = (i % FCC) * OUT
            for oc in range(3):
                ocn = OCN[oc]
                nc.tensor.matmul(
                    out=pfc[oc][0:ocn, 0:16],
                    lhsT=fw[:, fo + oc * 128: fo + oc * 128 + ocn],
                    rhs=poolv[:, cc, :, h],
                    start=(i == 0),
                    stop=(i == nhc - 1),
                )
    for oc in range(3):
        ocn = OCN[oc]
        nc.scalar.activation(
            out=fcout[0:ocn, oc * 16: oc * 16 + 16],
            in_=pfc[oc][0:ocn, 0:16],
            func=AF.Identity,
            bias=fcb[0:ocn, oc: oc + 1],
        )

    # ---- sim = fcout.T @ gloveT ; log_softmax over NA ----
    # rotate pss through BOTH free pools (3 banks) so chunk i+2's matmuls
    # don't wait on chunk i's exp/copy readers
    a0 = 0
    for ci, n in enumerate(SIMCH):
        pss = (psyt.tile([16, 512], F32, tag="ytps", name="pss")
               if ci % 2 == 0 else
               pstr.tile([16, 512], F32, tag="trps", name="pss2"))
        for oc in range(3):
            ocn = OCN[oc]
            nc.tensor.matmul(
                out=pss[0:16, 0:n],
                lhsT=fcout[0:ocn, oc * 16: oc * 16 + 16],
                rhs=glo_sb[oc][0:ocn, a0: a0 + n],
                start=(oc == 0),
                stop=(oc == 2),
            )
        junk = tmpp.tile([128, 512], F32, tag="tmp")
        nc.scalar.activation(junk[0:16, 0:n], pss[0:16, 0:n], AF.Exp,
                             accum_out=parti[:, ci: ci + 1])
        nc.vector.tensor_copy(sim_sb[:, a0: a0 + n], pss[0:16, 0:n])
        a0 += n

    nc.vector.reduce_sum(out=tot[:], in_=parti[:, 0:10], axis=AX.X)
    nc.scalar.activation(lse[:], tot[:], AF.Ln)
    nlse = smlp.tile([BPC, 1], F32, tag="nlse")
    nc.vector.tensor_scalar_mul(nlse[:], lse[:], -1.0)
    # chunked subtract + store, split across ACT and DVE so the halves run
    # in parallel and the output DMAs overlap the arithmetic
    a0 = 0
    for ci, n in enumerate(SIMCH):
        if ci % 2 == 0:
            nc.scalar.activation(sim_sb[:, a0:a0 + n], sim_sb[:, a0:a0 + n],
                                 AF.Identity, bias=nlse[:])
        else:
            nc.vector.tensor_scalar_sub(sim_sb[:, a0:a0 + n],
                                        sim_sb[:, a0:a0 + n], lse[:])
        nc.sync.dma_start(out_d[:, a0:a0 + n], sim_sb[:, a0:a0 + n])
        a0 += n


def _build():
    nc = bacc.Bacc("TRN2", target_bir_lowering=False, debug=False,
                   num_devices=NCORES, enable_partition_id=False)
    ins = {}

    def di(name, shape, dtype):
        ins[name] = nc.dram_tensor(name, list(shape), dtype,
                                   kind="ExternalInput").ap()

    di("all", (UMAX + 128 * 53982 // ES, ES), BF16)
    outs = {"out": nc.dram_tensor("out", [BPC, NA], F32,
                                  kind="ExternalOutput").ap()}

    with tile.TileContext(nc) as tc, ExitStack() as ctx:
        _emit(ctx, tc, ins, outs)
    nc.compile()
    return nc


def _pack_wT(W, bias):
    """[C, 900] f32 -> [128, NCH*1024] bf16 with bias folded at (0,2) row 44."""
    bf = ml_dtypes.bfloat16
    P = np.zeros((128, NCH * 1024), np.float32)
    for j in range(NODES):
        for c in range(3):
            ch = j * 3 + c
            kr = min(128, E - c * 128)
            P[0:kr, ch * 1024:(ch + 1) * 1024] = \
                W[:, j * E + c * 128: j * E + c * 128 + kr].T
    P[44, 2 * 1024:3 * 1024] = bias      # chunk (0,2) row 44 <- ones column
    return P.astype(bf)


def make_in_maps(he_ques, he_kg, emb, q2h_w, q2h_b, k2h_w, k2h_b,
                 h2att_w, h2att_b, fc_w, fc_b, glove_cands):
    f32 = np.float32
    bf = ml_dtypes.bfloat16
    emb = np.asarray(emb, f32)
    he_kg = np.asarray(he_kg).astype(np.int64)
    he_ques = np.asarray(he_ques).astype(np.int64)

    kwT = _pack_wT(np.asarray(k2h_w, f32), np.asarray(k2h_b, f32))
    qwT = _pack_wT(np.asarray(q2h_w, f32), np.asarray(q2h_b, f32))

    h2aT = np.zeros((128, CC * H), f32)
    for cc in range(CC):
        h2aT[:, cc * H:(cc + 1) * H] = np.asarray(h2att_w, f32)[:, cc * 128:(cc + 1) * 128].T

    fcb = np.zeros((128, 3), f32)
    fcb_src = np.asarray(fc_b, f32)
    for oc in range(3):
        fcb[0:OCN[oc], oc] = fcb_src[oc * 128: oc * 128 + OCN[oc]]

    sel1 = np.zeros((128, H), f32)
    for p in range(128):
        sel1[p, p // 16] = 1.0
    sel2 = np.ascontiguousarray(sel1.T)
    idbf = np.eye(128, dtype=bf)

    fcw = np.asarray(fc_w, f32).reshape(OUT, H, CC, 128)
    fcwT = np.ascontiguousarray(
        fcw.transpose(3, 1, 2, 0).reshape(128, H * CC * OUT)).astype(bf)

    glo = np.asarray(glove_cands, f32)
    gloT = np.zeros((3, 128, NA), f32)
    for oc in range(3):
        gloT[oc, 0:OCN[oc], :] = glo[:, oc * 128: oc * 128 + OCN[oc]].T
    gloT = gloT.astype(bf)

    # merged bf16 buffer: qwT | kwT | idbf | fcwT | glo0..2 | f32 consts
    # (bitcast) | per-core i16 idx appended later
    wf = np.zeros((128, 203), f32)
    wf[:, 0:64] = h2aT
    wf[:, 64:67] = fcb
    wf[:, 67:75] = sel1
    wf[0:H, 75:203] = sel2
    wb_common = np.ascontiguousarray(np.concatenate(
        [qwT, kwT, idbf, fcwT, gloT[0], gloT[1], gloT[2],
         np.ascontiguousarray(wf).view(bf)], axis=1))
    shared = {}

    def wrap_idx(flat):
        """[n] -> [128, n//16] int16 wrapped in 16 partitions, replicated."""
        n = flat.shape[0]
        t = np.zeros((128, n // 16), np.int16)
        t[0:16] = flat.reshape(n // 16, 16).T
        for g in range(1, 8):
            t[g * 16:(g + 1) * 16] = t[0:16]
        return t

    maps = []
    for core in range(NCORES):
        kg = he_kg[core * BPC:(core + 1) * BPC]       # [16, 256, 3]
        qu = he_ques[core * BPC:(core + 1) * BPC]     # [16, 16, 3]
        uniq, inv = np.unique(np.concatenate([kg.ravel(), qu.ravel()]),
                              return_inverse=True)
        assert len(uniq) <= UMAX
        kg_c = inv[:kg.size].reshape(kg.shape)
        qu_c = inv[kg.size:].reshape(qu.shape)

        emb_c = np.zeros((UMAX, ES), bf)
        emb_c[0:len(uniq), 0:E] = emb[uniq].astype(bf)
        emb_c[0:len(uniq), E] = bf(1.0)

        # kg idx per pair: i = j*512 + (half*256 + s)
        kg_flat = np.zeros((8, NODES * 512), np.int64)
        for bp in range(8):
            blk = kg_c[2 * bp:2 * bp + 2]             # [2, 256, 3]
            kg_flat[bp] = blk.transpose(2, 0, 1).reshape(NODES, 512).reshape(-1)
        kg_idx = np.concatenate([wrap_idx(kg_flat[bp]) for bp in range(8)],
                                axis=1).astype(np.int16)

        # q idx: i = j*256 + (b*16 + q)
        q_flat = qu_c.transpose(2, 0, 1).reshape(-1)
        q_idx = wrap_idx(q_flat)

        wi = np.ascontiguousarray(
            np.concatenate([q_idx, kg_idx], axis=1).astype(np.int16))

        wb_core = np.ascontiguousarray(
            np.concatenate([wb_common, wi.view(bf)], axis=1))
        m = dict(shared)
        m["all"] = np.ascontiguousarray(np.concatenate(
            [emb_c.reshape(-1), wb_core.reshape(-1)]).reshape(-1, ES))
        maps.append(m)
    return maps


def kernel(**inputs):
    global _CACHED
    if _CACHED is None:
        _CACHED = _build()
    nc = _CACHED
    in_maps = make_in_maps(**inputs)
    res = run_bass_kernel_spmd(nc, in_maps, list(range(NCORES)))
    return np.concatenate([r["out"] for r in res.results], axis=0)


# revision 29
# speedup vs baseline: 1.1254x; 1.1254x over previous
"""HAN (hypergraph attention network) Trainium2 kernel, v2.

Data-parallel over batch: 8 cores x 16 batch elements, params replicated.
v2 pipeline: per-core vocabulary compaction (int16 idx) + bf16 padded
embedding table with a built-in ones column -> dma_gather(transpose=True)
lands activations k-major with zero PE transposes -> bf16 projections with
bias folded into the weight row for the ones column -> bilinear attention
(heads*queries = 128 partitions) -> softmax -> bf16 attention-value matmul
-> pooled -> fc -> candidate sim -> log_softmax.  fc/glove weights are
loaded as a few large resident tiles so the tail phases never stall on DMA.
"""

import numpy as np
import ml_dtypes
from contextlib import ExitStack

import concourse.bass as bass
import concourse.bacc as bacc
import concourse.tile as tile
from concourse import mybir
from concourse.bass_utils import run_bass_kernel_spmd

F32 = mybir.dt.float32
BF16 = mybir.dt.bfloat16
I16 = mybir.dt.int16
AF = mybir.ActivationFunctionType
ALU = mybir.AluOpType
AX = mybir.AxisListType

NCORES = 8
B = 128
BPC = B // NCORES          # 16 batch elems per core
NQ, NS, NODES = 16, 256, 3
V, E = 50000, 300
ES = 384                   # padded emb row (bf16) -> 768B, %256==0
UMAX = 13056               # per-core unique rows cap: 16*(256+16)*3 / ... hard bound
C, H, OUT, NA = 1024, 8, 300, 5000
CC = C // 128              # 8 c-chunks
NCH = 9                    # (node j, k-chunk c) pairs: 3x3
OCN = [128, 128, 44]       # OUT=300 -> 3 o-chunks
SIMCH = [512] * 9 + [392]  # NA=5000 N-chunks
FCT = 4                    # fcw resident tiles
FCC = H * CC // FCT        # (h,cc) chunks per fcw tile

_CACHED = None


def _emit(ctx, tc, ins, outs):
    nc = tc.nc

    # single merged input (fewer per-call args = less dispatch overhead).
    # rows 0:UMAX = emb table (gather); rows UMAX: = a [128, 53982] bf16
    # param block flattened row-major, cols: 0:9216 qwT | 9216:18432 kwT |
    # 18432:18560 idbf | 18560:37760 fcwT | 37760:52760 glo | 52760:53166
    # f32 consts (h2aT|fcb|sel1|sel2, bitcast) | 53166:53982 i16 idx (q|kg)
    allt = ins["all"]           # [UMAX + 128*53982/ES, ES] bf16
    emb = allt[0:UMAX, :]
    wb = allt[UMAX:, :].rearrange("r c -> (r c)").rearrange(
        "(p x) -> p x", p=128)             # [128, 53982] bf16
    wf = wb[:, 52760:53166].bitcast(F32)   # [128, 203] f32
    wi = wb[:, 53166:53982].bitcast(I16)   # [128, 816] i16
    out_d = outs["out"]         # [BPC, NA] f32

    const = ctx.enter_context(tc.tile_pool(name="const", bufs=1))
    actp = ctx.enter_context(tc.tile_pool(name="actp", bufs=2))
    hstp = ctx.enter_context(tc.tile_pool(name="hstp", bufs=2))
    hsbp = ctx.enter_context(tc.tile_pool(name="hsbp", bufs=2))
    xtp = ctx.enter_context(tc.tile_pool(name="xtp", bufs=2))
    attp = ctx.enter_context(tc.tile_pool(name="attp", bufs=2))
    tmpp = ctx.enter_context(tc.tile_pool(name="tmpp", bufs=2))
    smlp = ctx.enter_context(tc.tile_pool(name="smlp", bufs=2))

    pstr = ctx.enter_context(tc.tile_pool(name="pstr", bufs=3, space="PSUM"))
    pspj = ctx.enter_context(tc.tile_pool(name="pspj", bufs=3, space="PSUM"))
    psyt = ctx.enter_context(tc.tile_pool(name="psyt", bufs=2, space="PSUM"))

    # ---- resident constants / weights (sliced out of the merged bufs) ----
    itq = const.tile([128, 48], I16, tag="itq")
    nc.sync.dma_start(itq[:], wi[:, 0:48])
    itk = const.tile([128, 8 * 96], I16, tag="itk")
    nc.sync.dma_start(itk[:], wi[:, 48:816])
    # qwT in 3 sub-tiles so the q-projection starts before the full load
    qwT3 = []
    for s in range(3):
        qt = const.tile([128, 3 * 1024], BF16, tag=f"qwT{s}")
        nc.sync.dma_start(qt[:], wb[:, s * 3072:(s + 1) * 3072])
        qwT3.append(qt)
    kwT3 = []
    for s in range(3):
        kt = const.tile([128, 3 * 1024], BF16, tag=f"kwT{s}")
        nc.sync.dma_start(kt[:], wb[:, 9216 + s * 3072: 9216 + (s + 1) * 3072])
        kwT3.append(kt)
    h2aT = const.tile([128, CC * H], F32, tag="h2aT")
    nc.sync.dma_start(h2aT[:], wf[:, 0:64])
    fcb = const.tile([128, 3], F32, tag="fcb")
    nc.sync.dma_start(fcb[:], wf[:, 64:67])
    sel1 = const.tile([128, H], F32, tag="sel1")
    nc.sync.dma_start(sel1[:], wf[:, 67:75])
    sel2 = const.tile([H, 128], F32, tag="sel2")
    nc.sync.dma_start(sel2[:], wf[0:H, 75:203])
    idbf = const.tile([128, 128], BF16, tag="idbf")
    nc.sync.dma_start(idbf[:], wb[:, 18432:18560])
    fcw_sb = []
    for t in range(FCT):
        fw = const.tile([128, FCC * OUT], BF16, tag=f"fcw{t}")
        nc.sync.dma_start(
            fw[:], wb[:, 18560 + t * FCC * OUT: 18560 + (t + 1) * FCC * OUT])
        fcw_sb.append(fw)
    glo_sb = []
    for oc in range(3):
        gs = const.tile([128, NA], BF16, tag=f"glo{oc}")
        nc.sync.dma_start(gs[:], wb[:, 37760 + oc * NA: 37760 + (oc + 1) * NA])
        glo_sb.append(gs)

    hqT = const.tile([128, CC * 256], F32, tag="hqT")      # [c, b*16+q]
    POOL = const.tile([128, CC * BPC * H], F32, tag="POOL")  # col cc*128+b*8+h
    POOLb = const.tile([128, CC * BPC * H], BF16, tag="POOLb")
    fcout = const.tile([128, 3 * BPC], BF16, tag="fcout")
    sim_sb = const.tile([BPC, NA], F32, tag="sim_sb")
    parti = const.tile([BPC, 16], F32, tag="parti")
    lse = const.tile([BPC, 1], F32, tag="lse")
    tot = const.tile([BPC, 1], F32, tag="tot")

    def gather(idx_slice, ntok):
        """ntok tokens x NODES rows, j-major idx order; num_idxs > 768
        crashes the gather ucode, so one 768-idx gather per contiguous
        [3, 768] block. Returns list of [128, 3, 768] views."""
        ni = NODES * ntok
        ng = ni // 768
        at = actp.tile([128, 3 * ni], BF16, tag="act")
        views = []
        for g in range(ng):
            v = at[:, g * 2304:(g + 1) * 2304].rearrange(
                "p (c n) -> p c n", c=3)
            nc.gpsimd.dma_gather(
                out_ap=v,
                in_ap=emb[:],
                idxs_ap=idx_slice[:, g * 48:(g + 1) * 48],
                num_idxs=768,
                num_idxs_reg=768,
                elem_size=ES,
                transpose=True,
            )
            views.append(v)
        return views

    def project(wT, atvs, dstT, ntok):
        """dstT[:, cc*ntok : +ntok] = wT.T @ act (+bias via ones column).

        rhs pieces per (node j, k-chunk c): with one gather (ntok=256) the
        j-blocks are whole; with two (ntok=512) node 1 straddles the two
        gather blocks, so it contributes two half-range matmuls."""
        if len(atvs) == 1:
            pieces = [(j * 3 + c, atvs[0][:, c, j * ntok:(j + 1) * ntok], 0, ntok)
                      for j in range(3) for c in range(3)]
        else:
            pieces = (
                [(c, atvs[0][:, c, 0:512], 0, 512) for c in range(3)]
                + [(3 + c, atvs[0][:, c, 512:768], 0, 256) for c in range(3)]
                + [(3 + c, atvs[1][:, c, 0:256], 256, 512) for c in range(3)]
                + [(6 + c, atvs[1][:, c, 256:768], 0, 512) for c in range(3)]
            )
            # full-range ops must open and close the accumulation group
            pieces = pieces[0:3] + pieces[3:9] + pieces[9:12]
        for cc in range(CC):
            ps = pspj.tile([128, 512], F32, tag="pjps")
            for i, (ch, rhs, a, bnd) in enumerate(pieces):
                if isinstance(wT, list):
                    w, col = wT[ch // 3], (ch % 3) * 1024 + cc * 128
                else:
                    w, col = wT, ch * 1024 + cc * 128
                nc.tensor.matmul(
                    out=ps[:, a:bnd],
                    lhsT=w[:, col: col + 128],
                    rhs=rhs,
                    start=(i == 0),
                    stop=(i == len(pieces) - 1),
                )
            nc.scalar.copy(out=dstT[:, cc * ntok:(cc + 1) * ntok],
                           in_=ps[:, 0:ntok])

    # ---- prologue: hq for all 16 b (256 ques tokens) ----
    atq = gather(itq[:], 256)
    project(qwT3, atq, hqT, 256)

    hqv = hqT[:].rearrange("p (c t) -> p c t", c=CC)  # [128, 8, 256]
    h2av = h2aT[:].rearrange("p (c h) -> p c h", c=CC)  # [128, 8, 8]
    pv = POOL[:].rearrange("p (c b h) -> p c b h", c=CC, b=BPC)
    pbv = POOLb[:].rearrange("p (c b h) -> p c b h", c=CC, b=BPC)

    # ---- per pair of batch elements ----
    for bp in range(BPC // 2):
        atk = gather(itk[:, bp * 96:(bp + 1) * 96], 512)

        hsT = hstp.tile([128, CC * 512], BF16, tag="hsT")
        project(kwT3, atk, hsT, 512)

        for half in range(2):
            b = bp * 2 + half
            hb = half * 256

            # hs token-major bf16: [s-chunk partitions, col st*1024 + c]
            hs_sb = hsbp.tile([128, 2 * 1024], BF16, tag="hs_sb")
            for st in range(2):
                ps = pstr.tile([128, 1024], BF16, tag="trps")
                for cc in range(CC):
                    nc.tensor.transpose(
                        out=ps[:, cc * 128:(cc + 1) * 128],
                        in_=hsT[:, cc * 512 + hb + st * 128: cc * 512 + hb + st * 128 + 128],
                        identity=idbf[:],
                    )
                nc.scalar.copy(out=hs_sb[:, st * 1024:(st + 1) * 1024], in_=ps[:])

            # X^T[c, h*16+q] = hqT[c, q] * h2aT[c, h]
            XT = xtp.tile([128, 1024], BF16, tag="XT")
            nc.vector.tensor_tensor(
                out=XT[:].rearrange("p (c h q) -> p c h q", c=CC, h=H),
                in0=hqv[:, :, b * 16: b * 16 + 16].unsqueeze(2).to_broadcast(
                    [128, CC, H, 16]),
                in1=h2av[:, :, :].unsqueeze(3).to_broadcast([128, CC, H, 16]),
                op=ALU.mult,
            )

            # logits[hq=128, s=256]
            plg = pspj.tile([128, 512], F32, tag="pjps")
            for cc in range(CC):
                nc.tensor.matmul(
                    out=plg[:, 0:256],
                    lhsT=XT[:, cc * 128: cc * 128 + 128],
                    rhs=hsT[:, cc * 512 + hb: cc * 512 + hb + 256],
                    start=(cc == 0),
                    stop=(cc == CC - 1),
                )

            # softmax over flat (q,s) per (b,h); logits tiny -> skip max-sub
            att = attp.tile([128, 256], BF16, tag="att")
            qsum = smlp.tile([128, 1], F32, tag="qsum")
            nc.scalar.activation(att[:], plg[:, 0:256], AF.Exp, accum_out=qsum[:])

            dps = psyt.tile([128, 512], F32, tag="ytps", name="dps")
            nc.tensor.matmul(out=dps[0:8, 0:1], lhsT=sel1[:], rhs=qsum[:],
                             start=True, stop=True)
            r8 = smlp.tile([8, 1], F32, tag="r8")
            nc.vector.reciprocal(r8[:], dps[0:8, 0:1])
            nc.tensor.matmul(out=dps[:, 1:2], lhsT=sel2[:], rhs=r8[:],
                             start=True, stop=True)
            rsb = smlp.tile([128, 1], F32, tag="rsb")
            nc.vector.tensor_copy(rsb[:], dps[:, 1:2])

            attn = attp.tile([128, 256], BF16, tag="attn")
            nc.vector.tensor_scalar_mul(attn[:], att[:], rsb[:])

            # attT [s, hq] bf16
            attT = attp.tile([128, 256], BF16, tag="attT")
            psTb = pstr.tile([128, 256], BF16, tag="trps", name="psTb")
            for st in range(2):
                nc.tensor.transpose(
                    out=psTb[:, st * 128:(st + 1) * 128],
                    in_=attn[:, st * 128:(st + 1) * 128],
                    identity=idbf[:],
                )
            nc.vector.tensor_copy(attT[:], psTb[:])

            # YT[c, hq] per c-chunk; pooled[h,c] = sum_q hqT * sum_s attT*hs
            for ccg in range(2):
                py = psyt.tile([128, 512], F32, tag="ytps")
                for i in range(4):
                    cc = ccg * 4 + i
                    for st in range(2):
                        nc.tensor.matmul(
                            out=py[:, i * 128:(i + 1) * 128],
                            lhsT=hs_sb[:, st * 1024 + cc * 128: st * 1024 + cc * 128 + 128],
                            rhs=attT[:, st * 128:(st + 1) * 128],
                            start=(st == 0),
                            stop=(st == 1),
                        )
                tmp = tmpp.tile([128, 512], F32, tag="tmp")
                nc.vector.tensor_tensor(
                    out=tmp[:].rearrange("p (c h q) -> p c h q", c=4, h=H),
                    in0=py[:].rearrange("p (c h q) -> p c h q", c=4, h=H),
                    in1=hqv[:, ccg * 4:(ccg + 1) * 4, b * 16: b * 16 + 16].unsqueeze(2).to_broadcast([128, 4, H, 16]),
                    op=ALU.mult,
                )
                nc.vector.reduce_sum(
                    out=pv[:, ccg * 4:(ccg + 1) * 4, b, :],
                    in_=tmp[:].rearrange("p (c h q) -> p c h q", c=4, h=H),
                    axis=AX.X,
                )
            # per-b bf16 conversion keeps the bulk copy off fc's critical path
            nc.vector.tensor_copy(pbv[:, :, b, :], pv[:, :, b, :])

    # ---- fc: out[o, b] = sum_{h,c} fc_w[o, h*1024+c] * pooled ----
    poolv = POOLb[:].rearrange("p (c b h) -> p c b h", c=CC, b=BPC)
    # pfc banks come from pools released by the projections (~175us), not
    # psyt which the last pair's YT holds until ~183us
    pfc = [pspj.tile([128, 512], F32, tag="pjps", name="pfc0"),
           pstr.tile([128, 512], F32, tag="trps", name="pfc1"),
           pspj.tile([128, 512], F32, tag="pjps", name="pfc2")]
    nhc = H * CC
    for h in range(H):
        for cc in range(CC):
            i = h * CC + cc
            fw = fcw_sb[i // FCC]
            fo = (i % FCC) * OUT
            for oc in range(3):
                ocn = OCN[oc]
                nc.tensor.matmul(
                    out=pfc[oc][0:ocn, 0:16],
                    lhsT=fw[:, fo + oc * 128: fo + oc * 128 + ocn],
                    rhs=poolv[:, cc, :, h],
                    start=(i == 0),
                    stop=(i == nhc - 1),
                )
    for oc in range(3):
        ocn = OCN[oc]
        nc.scalar.activation(
            out=fcout[0:ocn, oc * 16: oc * 16 + 16],
            in_=pfc[oc][0:ocn, 0:16],
            func=AF.Identity,
            bias=fcb[0:ocn, oc: oc + 1],
        )

    # ---- sim = fcout.T @ gloveT ; log_softmax over NA ----
    # rotate pss through BOTH free pools (3 banks) so chunk i+2's matmuls
    # don't wait on chunk i's exp/copy readers
    a0 = 0
    for ci, n in enumerate(SIMCH):
        pss = (psyt.tile([16, 512], F32, tag="ytps", name="pss")
               if ci % 2 == 0 else
               pstr.tile([16, 512], F32, tag="trps", name="pss2"))
        for oc in range(3):
            ocn = OCN[oc]
            nc.tensor.matmul(
                out=pss[0:16, 0:n],
                lhsT=fcout[0:ocn, oc * 16: oc * 16 + 16],
                rhs=glo_sb[oc][0:ocn, a0: a0 + n],
                start=(oc == 0),
                stop=(oc == 2),
            )
        junk = tmpp.tile([128, 512], F32, tag="tmp")
        nc.scalar.activation(junk[0:16, 0:n], pss[0:16, 0:n], AF.Exp,
                             accum_out=parti[:, ci: ci + 1])
        nc.vector.tensor_copy(sim_sb[:, a0: a0 + n], pss[0:16, 0:n])
        a0 += n

    nc.vector.reduce_sum(out=tot[:], in_=parti[:, 0:10], axis=AX.X)
    nc.scalar.activation(lse[:], tot[:], AF.Ln)
    nlse = smlp.tile([BPC, 1], F32, tag="nlse")
    nc.vector.tensor_scalar_mul(nlse[:], lse[:], -1.0)
    # chunked subtract + store, split across ACT and DVE so the halves run
    # in parallel and the output DMAs overlap the arithmetic
    a0 = 0
    for ci, n in enumerate(SIMCH):
        if ci % 2 == 0:
            nc.scalar.activation(sim_sb[:, a0:a0 + n], sim_sb[:, a0:a0 + n],
                                 AF.Identity, bias=nlse[:])
        else:
            nc.vector.tensor_scalar_sub(sim_sb[:, a0:a0 + n],
                                        sim_sb[:, a0:a0 + n], lse[:])
        nc.sync.dma_start(out_d[:, a0:a0 + n], sim_sb[:, a0:a0 + n])
        a0 += n


def _build():
    nc = bacc.Bacc("TRN2", target_bir_lowering=False, debug=False,
                   num_devices=NCORES, enable_partition_id=False)
    ins = {}

    def di(name, shape, dtype):
        ins[name] = nc.dram_tensor(name, list(shape), dtype,
                                   kind="ExternalInput").ap()

    di("all", (UMAX + 128 * 53982 // ES, ES), BF16)
    outs = {"out": nc.dram_tensor("out", [BPC, NA], F32,
                                  kind="ExternalOutput").ap()}

    with tile.TileContext(nc) as tc, ExitStack() as ctx:
        _emit(ctx, tc, ins, outs)
    nc.compile()
    return nc


def _pack_wT(W, bias):
    """[C, 900] f32 -> [128, NCH*1024] bf16 with bias folded at (0,2) row 44."""
    bf = ml_dtypes.bfloat16
    P = np.zeros((128, NCH * 1024), np.float32)
    for j in range(NODES):
        for c in range(3):
            ch = j * 3 + c
            kr = min(128, E - c * 128)
            P[0:kr, ch * 1024:(ch + 1) * 1024] = \
                W[:, j * E + c * 128: j * E + c * 128 + kr].T
    P[44, 2 * 1024:3 * 1024] = bias      # chunk (0,2) row 44 <- ones column
    return P.astype(bf)


def make_in_maps(he_ques, he_kg, emb, q2h_w, q2h_b, k2h_w, k2h_b,
                 h2att_w, h2att_b, fc_w, fc_b, glove_cands):
    f32 = np.float32
    bf = ml_dtypes.bfloat16
    emb = np.asarray(emb, f32)
    he_kg = np.asarray(he_kg).astype(np.int64)
    he_ques = np.asarray(he_ques).astype(np.int64)

    kwT = _pack_wT(np.asarray(k2h_w, f32), np.asarray(k2h_b, f32))
    qwT = _pack_wT(np.asarray(q2h_w, f32), np.asarray(q2h_b, f32))

    h2aT = np.zeros((128, CC * H), f32)
    for cc in range(CC):
        h2aT[:, cc * H:(cc + 1) * H] = np.asarray(h2att_w, f32)[:, cc * 128:(cc + 1) * 128].T

    fcb = np.zeros((128, 3), f32)
    fcb_src = np.asarray(fc_b, f32)
    for oc in range(3):
        fcb[0:OCN[oc], oc] = fcb_src[oc * 128: oc * 128 + OCN[oc]]

    sel1 = np.zeros((128, H), f32)
    for p in range(128):
        sel1[p, p // 16] = 1.0
    sel2 = np.ascontiguousarray(sel1.T)
    idbf = np.eye(128, dtype=bf)

    fcw = np.asarray(fc_w, f32).reshape(OUT, H, CC, 128)
    fcwT = np.ascontiguousarray(
        fcw.transpose(3, 1, 2, 0).reshape(128, H * CC * OUT)).astype(bf)

    glo = np.asarray(glove_cands, f32)
    gloT = np.zeros((3, 128, NA), f32)
    for oc in range(3):
        gloT[oc, 0:OCN[oc], :] = glo[:, oc * 128: oc * 128 + OCN[oc]].T
    gloT = gloT.astype(bf)

    # merged bf16 buffer: qwT | kwT | idbf | fcwT | glo0..2 | f32 consts
    # (bitcast) | per-core i16 idx appended later
    wf = np.zeros((128, 203), f32)
    wf[:, 0:64] = h2aT
    wf[:, 64:67] = fcb
    wf[:, 67:75] = sel1
    wf[0:H, 75:203] = sel2
    wb_common = np.ascontiguousarray(np.concatenate(
        [qwT, kwT, idbf, fcwT, gloT[0], gloT[1], gloT[2],
         np.ascontiguousarray(wf).view(bf)], axis=1))
    shared = {}

    def wrap_idx(flat):
        """[n] -> [128, n//16] int16 wrapped in 16 partitions, replicated."""
        n = flat.shape[0]
        t = np.zeros((128, n // 16), np.int16)
        t[0:16] = flat.reshape(n // 16, 16).T
        for g in range(1, 8):
            t[g * 16:(g + 1) * 16] = t[0:16]
        return t

    maps = []
    for core in range(NCORES):
        kg = he_kg[core * BPC:(core + 1) * BPC]       # [16, 256, 3]
        qu = he_ques[core * BPC:(core + 1) * BPC]     # [16, 16, 3]
        uniq, inv = np.unique(np.concatenate([kg.ravel(), qu.ravel()]),
                              return_inverse=True)
        assert len(uniq) <= UMAX
        kg_c = inv[:kg.size].reshape(kg.shape)
        qu_c = inv[kg.size:].reshape(qu.shape)

        emb_c = np.zeros((UMAX, ES), bf)
        emb_c[0:len(uniq), 0:E] = emb[uniq].astype(bf)
        emb_c[0:len(uniq), E] = bf(1.0)

        # kg idx per pair: i = j*512 + (half*256 + s)
        kg_flat = np.zeros((8, NODES * 512), np.int64)
        for bp in range(8):
            blk = kg_c[2 * bp:2 * bp + 2]             # [2, 256, 3]
            kg_flat[bp] = blk.transpose(2, 0, 1).reshape(NODES, 512).reshape(-1)
        kg_idx = np.concatenate([wrap_idx(kg_flat[bp]) for bp in range(8)],
                                axis=1).astype(np.int16)

        # q idx: i = j*256 + (b*16 + q)
        q_flat = qu_c.transpose(2, 0, 1).reshape(-1)
        q_idx = wrap_idx(q_flat)

        wi = np.ascontiguousarray(
            np.concatenate([q_idx, kg_idx], axis=1).astype(np.int16))

        wb_core = np.ascontiguousarray(
            np.concatenate([wb_common, wi.view(bf)], axis=1))
        m = dict(shared)
        m["all"] = np.ascontiguousarray(np.concatenate(
            [emb_c.reshape(-1), wb_core.reshape(-1)]).reshape(-1, ES))
        maps.append(m)
    return maps


def kernel(**inputs):
    global _CACHED
    if _CACHED is None:
        _CACHED = _build()
    nc = _CACHED
    in_maps = make_in_maps(**inputs)
    res = run_bass_kernel_spmd(nc, in_maps, list(range(NCORES)))
    return np.concatenate([r["out"] for r in res.results], axis=0)
